# revision 20
# baseline (speedup 1.0000x reference)
"""GQA attention block (QKV proj + RoPE + causal attention + o_proj),
tensor-parallel over heads across 8 TRN2 NeuronCores.

Sharding: core c owns q heads [4c, 4c+4) (512 q dims), kv head c
(128 kv dims), and wo columns [512c, 512c+512). Each core computes a
full-shape partial of the output projection; the host sums the 8
partials (the "all-reduce") and transposes back.

Layout convention on device: activations are kept feature-major
([dim, seq]) so every matmul contracts over the partition axis with
no transposes:
  QT/KT [d, s]  ->  scores^T [ks, qs] = KT_tile^T . QT   (lhsT=KT, rhs=QT)
  softmax over ks = partition axis: exp on ACT, denominator via
  ones-matmul on PE, division folded into the PV output scaling
  PV: OT [dv, qs] = V_nat^T . P                           (lhsT=V, rhs=P)
  o_proj: outT [e, s] = woT^T . OT                        (lhsT=woT, rhs=OT)
Matmuls run as float32r (full-rate fp32 on the PE for free dim >= 256).
"""

import sys
from contextlib import ExitStack

import numpy as np

for _p in ("/opt/trn_rl_repo", "/opt/trn_rl_repo/concourse"):
    if _p not in sys.path:
        sys.path.insert(0, _p)

import concourse.bacc as bacc
import concourse.bass as bass
import concourse.tile as tile
from concourse import mybir
from concourse.bass_utils import run_bass_kernel_spmd

F32 = mybir.dt.float32
F32R = mybir.dt.float32r
AF = mybir.ActivationFunctionType

DIM = 4096
SEQ = 2048
HD = 128          # head dim
NCORES = 8
HQ = 4            # q heads per core
DQ = HQ * HD      # 512 q dims per core
NKT = DIM // HD   # 32 contraction tiles
SQT = SEQ // 512  # 4 seq chunks of 512
INV_SQRT_HD = 1.0 / np.sqrt(np.float32(HD))
EXP_BIAS = -12.0  # constant shift inside exp; cancels in softmax

TRACE = False
LAST_RESULT = None

_cache = {}


def _build(mask_mode):
    """mask_mode: 'zeros' | 'causal' | 'general'."""
    nc = bacc.Bacc("TRN2", target_bir_lowering=False)
    xt = nc.dram_tensor("xt", [DIM, SEQ], F32R, kind="ExternalInput")
    wqt = nc.dram_tensor("wqt", [DIM, DQ], F32R, kind="ExternalInput")
    wkt = nc.dram_tensor("wkt", [DIM, HD], F32R, kind="ExternalInput")
    wvt = nc.dram_tensor("wvt", [DIM, HD], F32R, kind="ExternalInput")
    wot = nc.dram_tensor("wot", [DQ, DIM], F32R, kind="ExternalInput")
    cs = nc.dram_tensor("cs", [HD, SEQ], F32, kind="ExternalInput")
    sn = nc.dram_tensor("sn", [HD, SEQ], F32, kind="ExternalInput")
    psw = nc.dram_tensor("psw", [HD, HD], F32R, kind="ExternalInput")
    idn = nc.dram_tensor("idn", [HD, HD], F32, kind="ExternalInput")
    mkt = None
    if mask_mode != "zeros":
        mkt = nc.dram_tensor("mkt", [SEQ, SEQ], F32, kind="ExternalInput")
    outt = nc.dram_tensor("outt", [DIM, SEQ], F32, kind="ExternalOutput")

    with ExitStack() as ctx:
        tc = ctx.enter_context(tile.TileContext(nc))

        # ---- persistent pools ----
        const = ctx.enter_context(tc.tile_pool(name="const", bufs=1))
        ones_f32 = const.tile([HD, HD], F32, tag="ones32")
        nc.vector.memset(ones_f32[:], 1.0)
        ones_sb = const.tile([HD, HD], F32R, tag="ones")
        nc.scalar.activation(ones_sb[:], ones_f32[:], AF.Copy)
        ebias = const.tile([HD, 1], F32, tag="ebias")
        nc.vector.memset(ebias[:], EXP_BIAS)

        # ---- phase 1: QKV projection + RoPE ----
        p2stack = ctx.enter_context(ExitStack())  # closed manually later? no

        qkvpool = ctx.enter_context(tc.tile_pool(name="qkv", bufs=1))
        qrope = [qkvpool.tile([HD, SEQ], F32R, tag=f"qrope{h}", name=f"qrope{h}")
                 for h in range(HQ)]
        krope = qkvpool.tile([HD, SEQ], F32R, tag="krope")
        vnat = qkvpool.tile([HD, SEQ], F32R, tag="vnat")

        with ExitStack() as p1:
            wpool = p1.enter_context(tc.tile_pool(name="w1", bufs=1))
            wq_sb = wpool.tile([HD, NKT * DQ], F32R, tag="wq")
            wk_sb = wpool.tile([HD, NKT * HD], F32R, tag="wk")
            wv_sb = wpool.tile([HD, NKT * HD], F32R, tag="wv")
            cs_sb = wpool.tile([HD, SEQ], F32, tag="cs")
            sn_sb = wpool.tile([HD, SEQ], F32, tag="sn")
            psw_sb = wpool.tile([HD, HD], F32R, tag="psw")
            idn_sb = wpool.tile([HD, HD], F32, tag="idn")
            def emit_w_dma(kg):
                k0 = kg * 4
                if kg > 0:
                    nc.sync.dma_start(
                        wq_sb[:, k0 * DQ:(k0 + 4) * DQ]
                        .rearrange("p (k m) -> p k m", k=4),
                        wqt[k0 * HD:(k0 + 4) * HD, :]
                        .rearrange("(k p) m -> p k m", p=HD))
                nc.sync.dma_start(
                    wk_sb[:, k0 * HD:(k0 + 4) * HD]
                    .rearrange("p (k m) -> p k m", k=4),
                    wkt[k0 * HD:(k0 + 4) * HD, :]
                    .rearrange("(k p) m -> p k m", p=HD))
                nc.sync.dma_start(
                    wv_sb[:, k0 * HD:(k0 + 4) * HD]
                    .rearrange("p (k m) -> p k m", k=4),
                    wvt[k0 * HD:(k0 + 4) * HD, :]
                    .rearrange("(k p) m -> p k m", p=HD))
                if kg == 2:
                    nc.sync.dma_start(psw_sb[:], psw[:])
                    nc.sync.dma_start(idn_sb[:], idn[:])
                    nc.sync.dma_start(cs_sb[:], cs[:])
                    nc.sync.dma_start(sn_sb[:], sn[:])

            xpool = p1.enter_context(tc.tile_pool(name="xstream", bufs=4))
            rtmp = p1.enter_context(tc.tile_pool(name="rtmp", bufs=2))
            ps1 = p1.enter_context(tc.tile_pool(name="ps1", bufs=1, space="PSUM"))
            ps1q = p1.enter_context(tc.tile_pool(name="ps1q", bufs=4, space="PSUM"))
            ps1m = p1.enter_context(tc.tile_pool(name="ps1m", bufs=1, space="PSUM"))

            for st in range(SQT):
                ss = slice(st * 512, (st + 1) * 512)
                pq = [ps1q.tile([HD, 512], F32, tag="pq", name=f"pq{i}") for i in range(HQ)]
                pk = ps1.tile([HD, 512], F32, tag="pk")
                pv = ps1.tile([HD, 512], F32, tag="pv")
                for kg in range(NKT // 4):
                    if st == 0 and kg == 0:
                        nc.sync.dma_start(
                            wq_sb[:, 0:4 * DQ]
                            .rearrange("p (k m) -> p k m", k=4),
                            wqt[0:4 * HD, :]
                            .rearrange("(k p) m -> p k m", p=HD))
                    xquad = xpool.tile([HD, 4 * 512], F32R, tag="xt")
                    nc.sync.dma_start(
                        xquad[:].rearrange("p (k m) -> p k m", k=4),
                        xt[kg * 4 * HD:(kg + 1) * 4 * HD, ss]
                        .rearrange("(k p) m -> p k m", p=HD),
                    )
                    if st == 0:
                        emit_w_dma(kg)
                    for kj in range(4):
                        kt = kg * 4 + kj
                        xr = xquad[:, kj * 512:(kj + 1) * 512]
                        fl = dict(start=(kt == 0), stop=(kt == NKT - 1))
                        for mt in range(HQ):
                            nc.tensor.matmul(
                                pq[mt][:],
                                wq_sb[:, kt * DQ + mt * HD:
                                      kt * DQ + (mt + 1) * HD],
                                xr, **fl,
                            )
                        nc.tensor.matmul(
                            pk[:], wk_sb[:, kt * HD:(kt + 1) * HD],
                            xr, **fl,
                        )
                        nc.tensor.matmul(
                            pv[:], wv_sb[:, kt * HD:(kt + 1) * HD],
                            xr, **fl,
                        )

                # RoPE on the four q tiles (scale 1/sqrt(hd) folded into copy)
                for mt in range(HQ):
                    raw = rtmp.tile([HD, 512], F32R, tag="qraw")
                    if mt % 2 == 0:
                        nc.scalar.activation(raw[:], pq[mt][:], AF.Copy,
                                             scale=float(INV_SQRT_HD))
                    else:
                        nc.vector.tensor_scalar_mul(raw[:], pq[mt][:],
                                                    float(INV_SQRT_HD))
                    swp = ps1m.tile([HD, 512], F32, tag="psw")
                    nc.tensor.matmul(swp[:], psw_sb[:], raw[:],
                                     start=True, stop=True)
                    t1 = rtmp.tile([HD, 512], F32, tag="t1", bufs=1)
                    nc.vector.tensor_mul(t1[:], raw[:], cs_sb[:, ss])
                    t2 = rtmp.tile([HD, 512], F32, tag="t2", bufs=1)
                    nc.vector.tensor_mul(t2[:], swp[:], sn_sb[:, ss])
                    nc.vector.tensor_add(qrope[mt][:, ss], t1[:], t2[:])
                # RoPE on k (unscaled)
                raw = rtmp.tile([HD, 512], F32R, tag="qraw")
                nc.scalar.activation(raw[:], pk[:], AF.Copy)
                swp = ps1m.tile([HD, 512], F32, tag="psw")
                nc.tensor.matmul(swp[:], psw_sb[:], raw[:], start=True, stop=True)
                t1 = rtmp.tile([HD, 512], F32, tag="t1", bufs=1)
                nc.vector.tensor_mul(t1[:], raw[:], cs_sb[:, ss])
                t2 = rtmp.tile([HD, 512], F32, tag="t2", bufs=1)
                nc.vector.tensor_mul(t2[:], swp[:], sn_sb[:, ss])
                nc.vector.tensor_add(krope[:, ss], t1[:], t2[:])
                # v: copy out and transpose to [seq, dv] blocks
                vraw = rtmp.tile([HD, 512], F32, tag="vraw", bufs=1)
                nc.vector.tensor_copy(vraw[:], pv[:])
                for j in range(4):
                    vt = ps1m.tile([HD, HD], F32, tag="pvt")
                    nc.tensor.transpose(vt[:], vraw[:, j * HD:(j + 1) * HD],
                                        idn_sb[:])
                    blk = st * 4 + j
                    nc.scalar.activation(
                        vnat[:, blk * HD:(blk + 1) * HD], vt[:], AF.Copy)

        # ---- phase 2: attention ----
        with ExitStack() as p2:
            wopool = p2.enter_context(tc.tile_pool(name="wo", bufs=1))
            wo_sb = [wopool.tile([HD, DIM], F32R, tag=f"wo{d}", name=f"wo{d}")
                     for d in range(HQ)]
            wo_dma_emitted = [False]

            def emit_wo_dmas():
                if not wo_dma_emitted[0]:
                    wo_dma_emitted[0] = True
                    for d in range(HQ):
                        nc.sync.dma_start(wo_sb[d][:],
                                          wot[d * HD:(d + 1) * HD, :])

            otpool = p2.enter_context(tc.tile_pool(name="ot", bufs=1))
            ot_sb = [otpool.tile([HD, SEQ], F32R, tag=f"ot{h}", name=f"ot{h}")
                     for h in range(HQ)]

            with ExitStack() as patt:
                mpool = patt.enter_context(tc.tile_pool(name="mk", bufs=1))
                ppool = patt.enter_context(tc.tile_pool(name="pp", bufs=4))
                spool = patt.enter_context(tc.tile_pool(name="sp", bufs=2))
                ps2 = patt.enter_context(
                    tc.tile_pool(name="ps2", bufs=4, space="PSUM"))
                ps2a = patt.enter_context(
                    tc.tile_pool(name="ps2a", bufs=2, space="PSUM"))

                def kslist(qt):
                    if mask_mode == "causal":
                        return (list(range(4 * qt + 4)),
                                set(range(4 * qt, 4 * qt + 4)))
                    ks = list(range(16))
                    return ks, (set(ks) if mask_mode == "general" else set())

                def emit_masks(qt):
                    qs = slice(qt * 512, (qt + 1) * 512)
                    _, msk = kslist(qt)
                    out = {}
                    if mask_mode == "causal":
                        k0 = 4 * qt
                        mq = mpool.tile([HD, 4 * 512], F32,
                                        tag=f"mkq{qt % 2}", name=f"mkq{qt % 2}")
                        nc.sync.dma_start(
                            mq[:].rearrange("p (k m) -> p k m", k=4),
                            mkt[k0 * HD:(k0 + 4) * HD, qs]
                            .rearrange("(k p) m -> p k m", p=HD))
                        for j, kst in enumerate(range(k0, k0 + 4)):
                            out[kst] = mq[:, j * 512:(j + 1) * 512]
                        return out
                    for kst in sorted(msk):
                        m = mpool.tile([HD, 512], F32, tag=f"mk{kst}",
                                       name=f"mk{kst}")
                        nc.sync.dma_start(
                            m[:], mkt[kst * HD:(kst + 1) * HD, qs])
                        out[kst] = m
                    return out

                mk_maps = {}
                for qt in range(SQT - 1, -1, -1):
                    qs = slice(qt * 512, (qt + 1) * 512)
                    ks_list, masked = kslist(qt)
                    if qt not in mk_maps:
                        mk_maps[qt] = emit_masks(qt)
                    mk_sb = mk_maps[qt]
                    emit_wo_dmas()

                    for h in range(HQ):
                        if (h == 1 and mask_mode == "causal"
                                and qt - 1 >= 0 and qt - 1 not in mk_maps):
                            mk_maps[qt - 1] = emit_masks(qt - 1)
                        n = len(ks_list)
                        sps = [None] * n
                        pbs = [None] * n

                        def issue_st(i):
                            kst = ks_list[i]
                            sp = ps2.tile([HD, 512], F32, tag="pst")
                            nc.tensor.matmul(
                                sp[:],
                                krope[:, kst * HD:(kst + 1) * HD],
                                qrope[h][:, qs],
                                start=True, stop=True,
                            )
                            sps[i] = sp

                        def issue_exp(i):
                            kst = ks_list[i]
                            pb = ppool.tile([HD, 512], F32R, tag="pexp")
                            if kst in masked:
                                tmp = ppool.tile([HD, 512], F32, tag="padd", bufs=2)
                                nc.vector.tensor_add(
                                    tmp[:], sps[i][:], mk_sb[kst])
                                nc.scalar.activation(pb[:], tmp[:], AF.Exp,
                                                     bias=ebias[:])
                            else:
                                nc.scalar.activation(pb[:], sps[i][:], AF.Exp,
                                                     bias=ebias[:])
                            pbs[i] = pb

                        den = ps2a.tile([HD, 512], F32, tag="pden")
                        otp = ps2a.tile([HD, 512], F32, tag="pot")
                        for j in range(min(3, n)):
                            issue_st(j)
                        for i in range(n):
                            if i + 3 < n:
                                issue_st(i + 3)
                            issue_exp(i)
                            kst = ks_list[i]
                            fl = dict(start=(i == 0), stop=(i == n - 1))
                            pr = pbs[i][:]
                            nc.tensor.matmul(
                                den[:], ones_sb[:], pr, **fl)
                            nc.tensor.matmul(
                                otp[:],
                                vnat[:, kst * HD:(kst + 1) * HD],
                                pr, **fl)
                        inv = spool.tile([HD, 512], F32, tag="inv")
                        nc.vector.reciprocal(inv[:], den[:])
                        nc.vector.tensor_mul(ot_sb[h][:, qs], otp[:], inv[:])

            # ---- phase 3: output projection (partial over this core's dims)
            with ExitStack() as p3:
                ps3 = p3.enter_context(
                    tc.tile_pool(name="ps3", bufs=4, space="PSUM"))
                opool = p3.enter_context(tc.tile_pool(name="ostage", bufs=4))
                for st in range(SQT):
                    ss = slice(st * 512, (st + 1) * 512)
                    for eg in range(DIM // HD // 4):
                        ocp = opool.tile([HD, 4 * 512], F32, tag="ocp")
                        for ej in range(4):
                            et = eg * 4 + ej
                            po = ps3.tile([HD, 512], F32, tag="po")
                            for d in range(HQ):
                                nc.tensor.matmul(
                                    po[:],
                                    wo_sb[d][:, et * HD:(et + 1) * HD],
                                    ot_sb[d][:, ss],
                                    start=(d == 0), stop=(d == HQ - 1),
                                )
                            oslice = ocp[:, ej * 512:(ej + 1) * 512]
                            if ej % 2 == 0:
                                nc.scalar.activation(oslice, po[:], AF.Copy)
                            else:
                                nc.vector.tensor_copy(oslice, po[:])
                        if st == SQT - 1 and eg == DIM // HD // 4 - 1:
                            for ej in range(4):
                                et = eg * 4 + ej
                                nc.sync.dma_start(
                                    outt[et * HD:(et + 1) * HD, ss],
                                    ocp[:, ej * 512:(ej + 1) * 512])
                        else:
                            nc.sync.dma_start(
                                outt[eg * 4 * HD:(eg + 1) * 4 * HD, ss]
                                .rearrange("(e p) m -> p e m", p=HD),
                                ocp[:].rearrange("p (e m) -> p e m", e=4))

    nc.compile()
    return nc


def _prep_consts(freqs_cos, freqs_sin):
    cos = np.asarray(freqs_cos, dtype=np.float32)
    sin = np.asarray(freqs_sin, dtype=np.float32)
    C = np.empty((HD, SEQ), np.float32)
    S = np.empty((HD, SEQ), np.float32)
    C[0::2] = cos.T
    C[1::2] = cos.T
    S[0::2] = -sin.T
    S[1::2] = sin.T
    psw = np.zeros((HD, HD), np.float32)
    j = np.arange(0, HD, 2)
    psw[j + 1, j] = 1.0
    psw[j, j + 1] = 1.0
    idn = np.eye(HD, dtype=np.float32)
    return C, S, psw, idn


def _mask_mode(mask):
    if not mask.any():
        return "zeros"
    neg = mask.min()
    tril = np.tril(np.ones((SEQ, SEQ), dtype=bool))
    if neg <= -1e8 and not mask[tril].any() and np.all(mask[~tril] == neg):
        return "causal"
    return "general"


def kernel(x, wq, wk, wv, wo, freqs_cos, freqs_sin, mask, start_pos):
    global LAST_RESULT
    assert int(start_pos) == 0, "kernel hardcodes start_pos=0 (full prefill)"
    x = np.asarray(x, dtype=np.float32)
    wq = np.asarray(wq, dtype=np.float32)
    wk = np.asarray(wk, dtype=np.float32)
    wv = np.asarray(wv, dtype=np.float32)
    wo = np.asarray(wo, dtype=np.float32)
    mask = np.asarray(mask, dtype=np.float32)

    mode = _mask_mode(mask)
    if mode not in _cache:
        _cache[mode] = _build(mode)
    nc = _cache[mode]

    xt = np.ascontiguousarray(x.reshape(SEQ, DIM).T)
    C, S, psw, idn = _prep_consts(freqs_cos, freqs_sin)
    mkt = None
    if mode != "zeros":
        mkt = np.ascontiguousarray(mask.T)

    in_maps = []
    for c in range(NCORES):
        m = {
            "xt": xt,
            "wqt": np.ascontiguousarray(wq[c * DQ:(c + 1) * DQ, :].T),
            "wkt": np.ascontiguousarray(wk[c * HD:(c + 1) * HD, :].T),
            "wvt": np.ascontiguousarray(wv[c * HD:(c + 1) * HD, :].T),
            "wot": np.ascontiguousarray(wo[:, c * DQ:(c + 1) * DQ].T),
            "cs": C, "sn": S, "psw": psw, "idn": idn,
        }
        if mkt is not None:
            m["mkt"] = mkt
        in_maps.append(m)

    res = run_bass_kernel_spmd(nc, in_maps, core_ids=list(range(NCORES)),
                               trace=TRACE)
    LAST_RESULT = res
    acc = np.zeros((DIM, SEQ), dtype=np.float64)
    for c in range(NCORES):
        acc += res.results[c]["outt"]
    return np.ascontiguousarray(acc.T).astype(np.float32).reshape(1, SEQ, DIM)


# revision 21
# speedup vs baseline: 1.0215x; 1.0215x over previous
"""GQA attention block (QKV proj + RoPE + causal attention + o_proj),
tensor-parallel over heads across 8 TRN2 NeuronCores.

Sharding: core c owns q heads [4c, 4c+4) (512 q dims), kv head c
(128 kv dims), and wo columns [512c, 512c+512). Each core computes a
full-shape partial of the output projection; the host sums the 8
partials (the "all-reduce") and transposes back.

Layout convention on device: activations are kept feature-major
([dim, seq]) so every matmul contracts over the partition axis with
no transposes:
  QT/KT [d, s]  ->  scores^T [ks, qs] = KT_tile^T . QT   (lhsT=KT, rhs=QT)
  softmax over ks = partition axis: exp on ACT, denominator via
  ones-matmul on PE, division folded into the PV output scaling
  PV: OT [dv, qs] = V_nat^T . P                           (lhsT=V, rhs=P)
  o_proj: outT [e, s] = woT^T . OT                        (lhsT=woT, rhs=OT)
Matmuls run as float32r (full-rate fp32 on the PE for free dim >= 256).
"""

import sys
from contextlib import ExitStack

import numpy as np

for _p in ("/opt/trn_rl_repo", "/opt/trn_rl_repo/concourse"):
    if _p not in sys.path:
        sys.path.insert(0, _p)

import concourse.bacc as bacc
import concourse.bass as bass
import concourse.tile as tile
from concourse import mybir
from concourse.bass_utils import run_bass_kernel_spmd

F32 = mybir.dt.float32
F32R = mybir.dt.float32r
AF = mybir.ActivationFunctionType

DIM = 4096
SEQ = 2048
HD = 128          # head dim
NCORES = 8
HQ = 4            # q heads per core
DQ = HQ * HD      # 512 q dims per core
NKT = DIM // HD   # 32 contraction tiles
SQT = SEQ // 512  # 4 seq chunks of 512
INV_SQRT_HD = 1.0 / np.sqrt(np.float32(HD))
EXP_BIAS = -12.0  # constant shift inside exp; cancels in softmax

TRACE = False
LAST_RESULT = None

_cache = {}


def _build(mask_mode):
    """mask_mode: 'zeros' | 'causal' | 'general'."""
    nc = bacc.Bacc("TRN2", target_bir_lowering=False)
    xt = nc.dram_tensor("xt", [DIM, SEQ], F32R, kind="ExternalInput")
    wqt = nc.dram_tensor("wqt", [DIM, DQ], F32R, kind="ExternalInput")
    wkt = nc.dram_tensor("wkt", [DIM, HD], F32R, kind="ExternalInput")
    wvt = nc.dram_tensor("wvt", [DIM, HD], F32R, kind="ExternalInput")
    wot = nc.dram_tensor("wot", [DQ, DIM], F32R, kind="ExternalInput")
    cs = nc.dram_tensor("cs", [HD, SEQ], F32, kind="ExternalInput")
    sn = nc.dram_tensor("sn", [HD, SEQ], F32, kind="ExternalInput")
    psw = nc.dram_tensor("psw", [HD, HD], F32R, kind="ExternalInput")
    idn = nc.dram_tensor("idn", [HD, HD], F32, kind="ExternalInput")
    mkt = None
    if mask_mode != "zeros":
        mkt = nc.dram_tensor("mkt", [SEQ, SEQ], F32, kind="ExternalInput")
    outt = nc.dram_tensor("outt", [DIM, SEQ], F32, kind="ExternalOutput")

    with ExitStack() as ctx:
        tc = ctx.enter_context(tile.TileContext(nc))

        # ---- persistent pools ----
        const = ctx.enter_context(tc.tile_pool(name="const", bufs=1))
        ones_f32 = const.tile([HD, HD], F32, tag="ones32")
        nc.vector.memset(ones_f32[:], 1.0)
        ones_sb = const.tile([HD, HD], F32R, tag="ones")
        nc.scalar.activation(ones_sb[:], ones_f32[:], AF.Copy)
        ebias = const.tile([HD, 1], F32, tag="ebias")
        nc.vector.memset(ebias[:], EXP_BIAS)

        # ---- phase 1: QKV projection + RoPE ----
        p2stack = ctx.enter_context(ExitStack())  # closed manually later? no

        qkvpool = ctx.enter_context(tc.tile_pool(name="qkv", bufs=1))
        qrope = [qkvpool.tile([HD, SEQ], F32R, tag=f"qrope{h}", name=f"qrope{h}")
                 for h in range(HQ)]
        krope = qkvpool.tile([HD, SEQ], F32R, tag="krope")
        vnat = qkvpool.tile([HD, SEQ], F32R, tag="vnat")

        with ExitStack() as p1:
            wpool = p1.enter_context(tc.tile_pool(name="w1", bufs=1))
            wq_sb = wpool.tile([HD, NKT * DQ], F32R, tag="wq")
            wk_sb = wpool.tile([HD, NKT * HD], F32R, tag="wk")
            wv_sb = wpool.tile([HD, NKT * HD], F32R, tag="wv")
            cs_sb = wpool.tile([HD, SEQ], F32, tag="cs")
            sn_sb = wpool.tile([HD, SEQ], F32, tag="sn")
            psw_sb = wpool.tile([HD, HD], F32R, tag="psw")
            idn_sb = wpool.tile([HD, HD], F32, tag="idn")
            def emit_w_dma(kg):
                k0 = kg * 4
                if kg > 0:
                    nc.sync.dma_start(
                        wq_sb[:, k0 * DQ:(k0 + 4) * DQ]
                        .rearrange("p (k m) -> p k m", k=4),
                        wqt[k0 * HD:(k0 + 4) * HD, :]
                        .rearrange("(k p) m -> p k m", p=HD))
                nc.sync.dma_start(
                    wk_sb[:, k0 * HD:(k0 + 4) * HD]
                    .rearrange("p (k m) -> p k m", k=4),
                    wkt[k0 * HD:(k0 + 4) * HD, :]
                    .rearrange("(k p) m -> p k m", p=HD))
                nc.sync.dma_start(
                    wv_sb[:, k0 * HD:(k0 + 4) * HD]
                    .rearrange("p (k m) -> p k m", k=4),
                    wvt[k0 * HD:(k0 + 4) * HD, :]
                    .rearrange("(k p) m -> p k m", p=HD))
                if kg == 2:
                    nc.sync.dma_start(psw_sb[:], psw[:])
                    nc.sync.dma_start(idn_sb[:], idn[:])
                    nc.sync.dma_start(cs_sb[:], cs[:])
                    nc.sync.dma_start(sn_sb[:], sn[:])

            xpool = p1.enter_context(tc.tile_pool(name="xstream", bufs=4))
            rtmp = p1.enter_context(tc.tile_pool(name="rtmp", bufs=2))
            ps1 = p1.enter_context(tc.tile_pool(name="ps1", bufs=1, space="PSUM"))
            ps1q = p1.enter_context(tc.tile_pool(name="ps1q", bufs=4, space="PSUM"))
            ps1m = p1.enter_context(tc.tile_pool(name="ps1m", bufs=1, space="PSUM"))

            for st in range(SQT):
                ss = slice(st * 512, (st + 1) * 512)
                pq = [ps1q.tile([HD, 512], F32, tag="pq", name=f"pq{i}") for i in range(HQ)]
                pk = ps1.tile([HD, 512], F32, tag="pk")
                pv = ps1.tile([HD, 512], F32, tag="pv")
                for kg in range(NKT // 4):
                    if st == 0 and kg == 0:
                        nc.sync.dma_start(
                            wq_sb[:, 0:4 * DQ]
                            .rearrange("p (k m) -> p k m", k=4),
                            wqt[0:4 * HD, :]
                            .rearrange("(k p) m -> p k m", p=HD))
                    xquad = xpool.tile([HD, 4 * 512], F32R, tag="xt")
                    nc.sync.dma_start(
                        xquad[:].rearrange("p (k m) -> p k m", k=4),
                        xt[kg * 4 * HD:(kg + 1) * 4 * HD, ss]
                        .rearrange("(k p) m -> p k m", p=HD),
                    )
                    if st == 0:
                        emit_w_dma(kg)
                    for kj in range(4):
                        kt = kg * 4 + kj
                        xr = xquad[:, kj * 512:(kj + 1) * 512]
                        fl = dict(start=(kt == 0), stop=(kt == NKT - 1))
                        for mt in range(HQ):
                            nc.tensor.matmul(
                                pq[mt][:],
                                wq_sb[:, kt * DQ + mt * HD:
                                      kt * DQ + (mt + 1) * HD],
                                xr, **fl,
                            )
                        nc.tensor.matmul(
                            pk[:], wk_sb[:, kt * HD:(kt + 1) * HD],
                            xr, **fl,
                        )
                        nc.tensor.matmul(
                            pv[:], wv_sb[:, kt * HD:(kt + 1) * HD],
                            xr, **fl,
                        )

                # RoPE on the four q tiles (scale 1/sqrt(hd) folded into copy)
                for mt in range(HQ):
                    raw = rtmp.tile([HD, 512], F32R, tag="qraw")
                    if mt % 2 == 0:
                        nc.scalar.activation(raw[:], pq[mt][:], AF.Copy,
                                             scale=float(INV_SQRT_HD))
                    else:
                        nc.vector.tensor_scalar_mul(raw[:], pq[mt][:],
                                                    float(INV_SQRT_HD))
                    swp = ps1m.tile([HD, 512], F32, tag="psw")
                    nc.tensor.matmul(swp[:], psw_sb[:], raw[:],
                                     start=True, stop=True)
                    t1 = rtmp.tile([HD, 512], F32, tag="t1", bufs=1)
                    nc.vector.tensor_mul(t1[:], raw[:], cs_sb[:, ss])
                    t2 = rtmp.tile([HD, 512], F32, tag="t2", bufs=1)
                    nc.vector.tensor_mul(t2[:], swp[:], sn_sb[:, ss])
                    nc.vector.tensor_add(qrope[mt][:, ss], t1[:], t2[:])
                # RoPE on k (unscaled)
                raw = rtmp.tile([HD, 512], F32R, tag="qraw")
                nc.scalar.activation(raw[:], pk[:], AF.Copy)
                swp = ps1m.tile([HD, 512], F32, tag="psw")
                nc.tensor.matmul(swp[:], psw_sb[:], raw[:], start=True, stop=True)
                t1 = rtmp.tile([HD, 512], F32, tag="t1", bufs=1)
                nc.vector.tensor_mul(t1[:], raw[:], cs_sb[:, ss])
                t2 = rtmp.tile([HD, 512], F32, tag="t2", bufs=1)
                nc.vector.tensor_mul(t2[:], swp[:], sn_sb[:, ss])
                nc.vector.tensor_add(krope[:, ss], t1[:], t2[:])
                # v: copy out and transpose to [seq, dv] blocks
                vraw = rtmp.tile([HD, 512], F32, tag="vraw", bufs=1)
                nc.vector.tensor_copy(vraw[:], pv[:])
                for j in range(4):
                    vt = ps1m.tile([HD, HD], F32, tag="pvt")
                    nc.tensor.transpose(vt[:], vraw[:, j * HD:(j + 1) * HD],
                                        idn_sb[:])
                    blk = st * 4 + j
                    nc.scalar.activation(
                        vnat[:, blk * HD:(blk + 1) * HD], vt[:], AF.Copy)

        # ---- phase 2: attention ----
        with ExitStack() as p2:
            wopool = p2.enter_context(tc.tile_pool(name="wo", bufs=1))
            wo_sb = [wopool.tile([HD, DIM], F32R, tag=f"wo{d}", name=f"wo{d}")
                     for d in range(HQ)]
            wo_dma_emitted = [False]

            def emit_wo_dmas():
                if not wo_dma_emitted[0]:
                    wo_dma_emitted[0] = True
                    for d in range(HQ):
                        nc.sync.dma_start(wo_sb[d][:],
                                          wot[d * HD:(d + 1) * HD, :])

            otpool = p2.enter_context(tc.tile_pool(name="ot", bufs=1))
            ot_sb = [otpool.tile([HD, SEQ], F32R, tag=f"ot{h}", name=f"ot{h}")
                     for h in range(HQ)]

            with ExitStack() as patt:
                mpool = patt.enter_context(tc.tile_pool(name="mk", bufs=1))
                ppool = patt.enter_context(tc.tile_pool(name="pp", bufs=4))
                spool = patt.enter_context(tc.tile_pool(name="sp", bufs=2))
                ps2 = patt.enter_context(
                    tc.tile_pool(name="ps2", bufs=4, space="PSUM"))
                ps2a = patt.enter_context(
                    tc.tile_pool(name="ps2a", bufs=2, space="PSUM"))

                def kslist(qt):
                    if mask_mode == "causal":
                        return (list(range(4 * qt + 4)),
                                set(range(4 * qt, 4 * qt + 4)))
                    ks = list(range(16))
                    return ks, (set(ks) if mask_mode == "general" else set())

                def emit_masks(qt):
                    qs = slice(qt * 512, (qt + 1) * 512)
                    _, msk = kslist(qt)
                    out = {}
                    if mask_mode == "causal":
                        k0 = 4 * qt
                        mq = mpool.tile([HD, 4 * 512], F32,
                                        tag=f"mkq{qt % 2}", name=f"mkq{qt % 2}")
                        nc.sync.dma_start(
                            mq[:].rearrange("p (k m) -> p k m", k=4),
                            mkt[k0 * HD:(k0 + 4) * HD, qs]
                            .rearrange("(k p) m -> p k m", p=HD))
                        for j, kst in enumerate(range(k0, k0 + 4)):
                            out[kst] = mq[:, j * 512:(j + 1) * 512]
                        return out
                    for kst in sorted(msk):
                        m = mpool.tile([HD, 512], F32, tag=f"mk{kst}",
                                       name=f"mk{kst}")
                        nc.sync.dma_start(
                            m[:], mkt[kst * HD:(kst + 1) * HD, qs])
                        out[kst] = m
                    return out

                mk_maps = {}
                for qt in range(SQT - 1, -1, -1):
                    qs = slice(qt * 512, (qt + 1) * 512)
                    ks_list, masked = kslist(qt)
                    if qt not in mk_maps:
                        mk_maps[qt] = emit_masks(qt)
                    mk_sb = mk_maps[qt]
                    emit_wo_dmas()

                    for h in range(HQ):
                        if (h == 1 and mask_mode == "causal"
                                and qt - 1 >= 0 and qt - 1 not in mk_maps):
                            mk_maps[qt - 1] = emit_masks(qt - 1)
                        n = len(ks_list)
                        sps = [None] * n
                        pbs = [None] * n

                        def cs0_of(kst):
                            # causal diag tile j: cols < j*128 fully masked
                            if mask_mode == "causal" and kst in masked:
                                return (kst - 4 * qt) * HD
                            return 0

                        def issue_st(i):
                            kst = ks_list[i]
                            c0 = cs0_of(kst)
                            sp = ps2.tile([HD, 512], F32, tag="pst")
                            nc.tensor.matmul(
                                sp[:, c0:],
                                krope[:, kst * HD:(kst + 1) * HD],
                                qrope[h][:, qt * 512 + c0:(qt + 1) * 512],
                                start=True, stop=True,
                            )
                            sps[i] = sp

                        def issue_exp(i):
                            kst = ks_list[i]
                            pb = ppool.tile([HD, 512], F32R, tag="pexp")
                            if kst in masked:
                                c0 = cs0_of(kst)
                                tmp = ppool.tile([HD, 512], F32, tag="padd", bufs=2)
                                nc.vector.tensor_add(
                                    tmp[:, c0:], sps[i][:, c0:],
                                    mk_sb[kst][:, c0:] if c0 else mk_sb[kst])
                                nc.scalar.activation(pb[:, c0:], tmp[:, c0:],
                                                     AF.Exp, bias=ebias[:])
                            else:
                                nc.scalar.activation(pb[:], sps[i][:], AF.Exp,
                                                     bias=ebias[:])
                            pbs[i] = pb

                        den = ps2a.tile([HD, 512], F32, tag="pden")
                        otp = ps2a.tile([HD, 512], F32, tag="pot")
                        for j in range(min(3, n)):
                            issue_st(j)
                        for i in range(n):
                            if i + 3 < n:
                                issue_st(i + 3)
                            issue_exp(i)
                            kst = ks_list[i]
                            c0 = cs0_of(kst)
                            fl = dict(start=(i == 0), stop=(i == n - 1))
                            pr = pbs[i][:, c0:]
                            nc.tensor.matmul(
                                den[:, c0:], ones_sb[:], pr, **fl)
                            nc.tensor.matmul(
                                otp[:, c0:],
                                vnat[:, kst * HD:(kst + 1) * HD],
                                pr, **fl)
                        inv = spool.tile([HD, 512], F32, tag="inv")
                        nc.vector.reciprocal(inv[:], den[:])
                        nc.vector.tensor_mul(ot_sb[h][:, qs], otp[:], inv[:])

            # ---- phase 3: output projection (partial over this core's dims)
            with ExitStack() as p3:
                ps3 = p3.enter_context(
                    tc.tile_pool(name="ps3", bufs=4, space="PSUM"))
                opool = p3.enter_context(tc.tile_pool(name="ostage", bufs=4))
                for st in range(SQT):
                    ss = slice(st * 512, (st + 1) * 512)
                    for eg in range(DIM // HD // 4):
                        ocp = opool.tile([HD, 4 * 512], F32, tag="ocp")
                        for ej in range(4):
                            et = eg * 4 + ej
                            po = ps3.tile([HD, 512], F32, tag="po")
                            for d in range(HQ):
                                nc.tensor.matmul(
                                    po[:],
                                    wo_sb[d][:, et * HD:(et + 1) * HD],
                                    ot_sb[d][:, ss],
                                    start=(d == 0), stop=(d == HQ - 1),
                                )
                            oslice = ocp[:, ej * 512:(ej + 1) * 512]
                            if ej % 2 == 0:
                                nc.scalar.activation(oslice, po[:], AF.Copy)
                            else:
                                nc.vector.tensor_copy(oslice, po[:])
                        if st == SQT - 1 and eg == DIM // HD // 4 - 1:
                            for ej in range(4):
                                et = eg * 4 + ej
                                nc.sync.dma_start(
                                    outt[et * HD:(et + 1) * HD, ss],
                                    ocp[:, ej * 512:(ej + 1) * 512])
                        else:
                            nc.sync.dma_start(
                                outt[eg * 4 * HD:(eg + 1) * 4 * HD, ss]
                                .rearrange("(e p) m -> p e m", p=HD),
                                ocp[:].rearrange("p (e m) -> p e m", e=4))

    nc.compile()
    return nc


def _prep_consts(freqs_cos, freqs_sin):
    cos = np.asarray(freqs_cos, dtype=np.float32)
    sin = np.asarray(freqs_sin, dtype=np.float32)
    C = np.empty((HD, SEQ), np.float32)
    S = np.empty((HD, SEQ), np.float32)
    C[0::2] = cos.T
    C[1::2] = cos.T
    S[0::2] = -sin.T
    S[1::2] = sin.T
    psw = np.zeros((HD, HD), np.float32)
    j = np.arange(0, HD, 2)
    psw[j + 1, j] = 1.0
    psw[j, j + 1] = 1.0
    idn = np.eye(HD, dtype=np.float32)
    return C, S, psw, idn


def _mask_mode(mask):
    if not mask.any():
        return "zeros"
    neg = mask.min()
    tril = np.tril(np.ones((SEQ, SEQ), dtype=bool))
    if neg <= -1e8 and not mask[tril].any() and np.all(mask[~tril] == neg):
        return "causal"
    return "general"


def kernel(x, wq, wk, wv, wo, freqs_cos, freqs_sin, mask, start_pos):
    global LAST_RESULT
    assert int(start_pos) == 0, "kernel hardcodes start_pos=0 (full prefill)"
    x = np.asarray(x, dtype=np.float32)
    wq = np.asarray(wq, dtype=np.float32)
    wk = np.asarray(wk, dtype=np.float32)
    wv = np.asarray(wv, dtype=np.float32)
    wo = np.asarray(wo, dtype=np.float32)
    mask = np.asarray(mask, dtype=np.float32)

    mode = _mask_mode(mask)
    if mode not in _cache:
        _cache[mode] = _build(mode)
    nc = _cache[mode]

    xt = np.ascontiguousarray(x.reshape(SEQ, DIM).T)
    C, S, psw, idn = _prep_consts(freqs_cos, freqs_sin)
    mkt = None
    if mode != "zeros":
        mkt = np.ascontiguousarray(mask.T)

    in_maps = []
    for c in range(NCORES):
        m = {
            "xt": xt,
            "wqt": np.ascontiguousarray(wq[c * DQ:(c + 1) * DQ, :].T),
            "wkt": np.ascontiguousarray(wk[c * HD:(c + 1) * HD, :].T),
            "wvt": np.ascontiguousarray(wv[c * HD:(c + 1) * HD, :].T),
            "wot": np.ascontiguousarray(wo[:, c * DQ:(c + 1) * DQ].T),
            "cs": C, "sn": S, "psw": psw, "idn": idn,
        }
        if mkt is not None:
            m["mkt"] = mkt
        in_maps.append(m)

    res = run_bass_kernel_spmd(nc, in_maps, core_ids=list(range(NCORES)),
                               trace=TRACE)
    LAST_RESULT = res
    acc = np.zeros((DIM, SEQ), dtype=np.float64)
    for c in range(NCORES):
        acc += res.results[c]["outt"]
    return np.ascontiguousarray(acc.T).astype(np.float32).reshape(1, SEQ, DIM)
